# revision 52
# baseline (speedup 1.0000x reference)
"""Trainium2 Bass kernel for the BiDirectionalRNN problem.

Math (matches the fp32 jax reference):
    e = emb[x]                                   # [B, T, 512]
    fwd:  h_t = relu(e_t @ Wf.T + bf + h_{t-1})  # fs[t]
    bwd over reversed e: bs[s]                   # generation order
    xcat = concat_t [fs[t], bs[t]]  -> [B, T*1024]
    h1 = relu(xcat @ W1.T + b1); 4x h = relu(h @ W2.T + b2); out = h @ Wo.T + bo

Strategy:
  * Data-parallel over batch: 1024/8 = 128 samples per NeuronCore.
  * Host folds embedding + input projection weights into per-direction
    tables WfeB = Wf @ emb.T + bf ([512, 97]). The device builds the
    one-hot of x on the fly (a rank-1 matmul replicates the x row over 97
    partitions, DVE is_equal against an arange column), then computes the
    per-step drive terms a = WfeB @ onehot with K=97 matmuls.
  * ScalarE copies each a-GEMM PSUM block into the scan layout
    [p, b*33 + s] (strided 3D AP), separator column = -1e30.
  * The whole 32-step recurrence h = relu(a + h_prev) runs as ONE DVE
    tensor_tensor_scan per (dir, hid-tile): state=(a add state) max 0,
    fp32 internal state; the separator resets state to 0 between chains.
  * W1 (64MB fp32) ships as bf16 (32MB/core) in 64 [128,2048] tiles,
    ordered (dir, m)-major so the GEMM can start right after the first
    scan; a-phases are software-pipelined two steps ahead of the groups
    that consume them. The [B,32768]@[32768,512] GEMM runs FEATURE-MAJOR:
    out[f, b] accumulates in one PSUM bank laid out as 4 f-tile column
    regions of 128 batch; lhsT = W1 k-chunk tiles [128k, 128f], rhs =
    strided scan-output views [128k, 128b]. This orientation needs no
    transposes at the tail and allows per-partition biases.
  * Tail: 4 x [512,512] layers + [97,512] head, all feature-major,
    pipelined in two 64-sample batch waves with per-wave activation tiles
    (byte-disjoint, no false deps): wave 1's matmuls run on PE while wave
    0's PSUM banks drain, so PE stays saturated through the tail. Every
    PSUM stage uses twin banks so ScalarE and VectorE drain in parallel
    (Tile serializes same-bank readers); biases enter PSUM via rank-1
    matmuls that open each accumulation group.
  * All const/small inputs ride in merged DMAs; the first two W1 groups
    are issued before them and the tail-only W2/Wo weights ship after the
    W1 stream (split so each piece's semaphore lands just before its
    layer matmuls), keeping the DMA engines gapless from first byte to
    the critical last W1 byte; the final four W1 groups are fetched in
    tapering chunk-aligned pieces to minimize the end latency.
"""

import numpy as np
import ml_dtypes

BF16 = ml_dtypes.bfloat16

MOD = 97
HID = 512
T = 32
B = 1024
NCORES = 8
BL = B // NCORES          # 128 batch per core
CL = T + 1                # chain length incl. separator column
FREE = BL * CL            # 4224 scan columns per tile
NEG = -1e30
W1_GRP = 64               # W1 DMA groups of 4 k-chunks (512KB each)
WAVE = 64                 # tail batch-wave size
WAVE0 = 64                # first-wave width (second wave = 128 - WAVE0)

_CACHE: dict = {}


def _build(reps=1):
    import concourse.tile as tile
    from concourse import bacc, mybir

    fp32 = mybir.dt.float32
    bf16 = mybir.dt.bfloat16

    nc = bacc.Bacc(
        "TRN2", target_bir_lowering=False, debug=False, num_devices=NCORES
    )

    d = {
        "WFE": nc.dram_tensor("WFE", [MOD, 2 * HID], bf16, kind="ExternalInput").ap(),
        "W1S": nc.dram_tensor("W1S", [W1_GRP, 128, 2048], bf16, kind="ExternalInput").ap(),
        "W2O": nc.dram_tensor("W2O", [128, 4 * 512 + 4 * MOD], bf16, kind="ExternalInput").ap(),
        "BIA": nc.dram_tensor("BIA", [1, 2 * BL * T + 1121], bf16, kind="ExternalInput").ap(),
        "COL": nc.dram_tensor("COL", [128, 16], fp32, kind="ExternalInput").ap(),
        "OUT": nc.dram_tensor("OUT", [MOD, BL], fp32, kind="ExternalOutput").ap(),
    }

    with tile.TileContext(nc) as tc:
        for _ in range(reps):
            _emit(tc, d, mybir)

    nc.compile()
    return nc


def _emit(tc, d, mybir):
    nc = tc.nc
    fp32 = mybir.dt.float32
    bf16 = mybir.dt.bfloat16
    AF = mybir.ActivationFunctionType
    ALU = mybir.AluOpType

    from contextlib import ExitStack

    with ExitStack() as ctx:
        const = ctx.enter_context(tc.tile_pool(name="const", bufs=1))
        a_pool = ctx.enter_context(tc.tile_pool(name="apool", bufs=2))
        h_pool = ctx.enter_context(tc.tile_pool(name="hpool", bufs=3))
        w1_pool = ctx.enter_context(tc.tile_pool(name="w1pool", bufs=24))
        hp_pool = ctx.enter_context(tc.tile_pool(name="hppool", bufs=5))
        ps_a = ctx.enter_context(tc.tile_pool(name="psa", bufs=2, space="PSUM"))
        ps_h1 = ctx.enter_context(tc.tile_pool(name="psh1", bufs=1, space="PSUM"))
        ps_l = ctx.enter_context(tc.tile_pool(name="psl", bufs=1, space="PSUM"))

        # ---- head: start the W1 stream before anything else ----
        # HWDGE descriptor generations serialize (~0.6us each); issuing the
        # first two W1 groups first keeps the DMA engines busy while the
        # const descriptors generate (W1 g0 isn't consumed until ~12us).
        w1_pre = {}
        for G in (0, 1):
            w_t = w1_pool.tile([128, 2048], bf16, tag="w_t")
            nc.sync.dma_start(w_t[:], d["W1S"][G])
            w1_pre[G] = w_t

        # ---- constants (merged DMAs to avoid early DMA-engine bubbles) ----
        wfe = const.tile([MOD, 2 * HID], bf16)
        nc.sync.dma_start(wfe[:], d["WFE"][:])
        bia = const.tile([1, 2 * BL * T + 1121], bf16)
        nc.sync.dma_start(bia[:], d["BIA"])
        xr = bia[:, 0:2 * BL * T]
        b1r = bia[:, 2 * BL * T:2 * BL * T + 512]
        b2r = bia[:, 2 * BL * T + 512:2 * BL * T + 1024]
        bor = bia[:, 2 * BL * T + 1024:2 * BL * T + 1121]
        col = const.tile([128, 16], fp32)
        nc.sync.dma_start(col[:], d["COL"])
        arn = col[:, 0:1]
        w2o = const.tile([128, 4 * 512 + 4 * MOD], bf16)
        w2sb = w2o[:, 0:2048]
        wosb = w2o[:, 2048:2048 + 4 * MOD]
        ones = const.tile([1, 128], bf16)
        nc.vector.memset(ones[:], 1.0)
        zero = const.tile([128, 1], bf16)
        nc.vector.memset(zero[:], 0.0)
        # one-hot of x, built on device: replicate the x row over 97
        # partitions with a rank-1 matmul, then compare against arange
        ohall = const.tile([MOD, 2 * BL * T], bf16)
        ohsb = [ohall[:, 0:BL * T], ohall[:, BL * T:2 * BL * T]]

        # ---- drive terms + scans + linear1, interleaved per j = dir*4 + m ----
        # a = WfeB @ onehot in 8 PSUM blocks of 16 chains; ScalarE lays each
        # block into the scan layout [p, b*33 + s] (strided 3D AP); the DVE
        # scan computes h = relu(a + h_prev) for all 128 chains in one
        # instruction; then the two W1 groups for this j stream in and
        # accumulate into psum_h1 (feature-major: 4 f-tile col regions).
        # W1 group order is (dir, m)-major so group G only needs scan j = G//8.
        # Two PSUM banks (f-tiles 0-1 / 2-3) so ScalarE and VectorE can drain
        # in parallel (Tile serializes same-bank readers).
        ph1a = ps_h1.tile([128, 256], fp32, tag="h1a")
        ph1b = ps_h1.tile([128, 256], fp32, tag="h1b")
        ph1 = [ph1a, ph1b]
        bias_done = [False]

        def a_phase(j):
            dd, m = j // 4, j % 4
            a_sb = a_pool.tile([128, FREE], bf16, tag="a")
            sep = a_sb[:].rearrange("p (b t) -> p b t", t=CL)[:, :, T]
            nc.vector.memset(sep, NEG)
            lhsT = wfe[:, dd * HID + m * 128: dd * HID + m * 128 + 128]
            for q in range(8):
                if m == 0:
                    px = ps_a.tile([128, 512], fp32, tag="pa")
                    nc.tensor.matmul(
                        px[:MOD, :], ones[:, 0:MOD],
                        xr[:, dd * BL * T + q * 512: dd * BL * T + (q + 1) * 512],
                        start=True, stop=True,
                    )
                    nc.vector.tensor_tensor(
                        ohsb[dd][:, q * 512:(q + 1) * 512], px[:MOD, :],
                        arn[:MOD, :].broadcast_to([MOD, 512]),
                        op=mybir.AluOpType.is_equal,
                    )
                pa = ps_a.tile([128, 512], fp32, tag="pa")
                nc.tensor.matmul(
                    pa[:], lhsT, ohsb[dd][:, q * 512:(q + 1) * 512],
                    start=True, stop=True,
                )
                av = a_sb[:].rearrange("p (b t) -> p b t", t=CL)[:, 16 * q:16 * q + 16, 0:T]
                nc.scalar.copy(av, pa[:].rearrange("p (b t) -> p b t", t=T))
            h_t = h_pool.tile([128, FREE], bf16, tag="h")
            nc.vector.tensor_tensor_scan(
                h_t[:], a_sb[:], zero[:].broadcast_to([128, FREE]),
                initial=0.0, op0=ALU.add, op1=ALU.max,
            )
            return h_t

        hs = {0: a_phase(0), 1: a_phase(1)}
        for j in range(8):
            hv = hs[j][:].rearrange("p (b t) -> p t b", t=CL)
            for G in range(8 * j, 8 * j + 8):
                w_t = w1_pre.pop(G, None)
                if w_t is None:
                    w_t = w1_pool.tile([128, 2048], bf16, tag="w_t")
                last_grp = G == W1_GRP - 1
                if G >= W1_GRP - 5:
                    # taper: fetch the final two groups in chunk-aligned
                    # pieces so each matmul only waits on its own slice and
                    # the post-stream PE backlog stays tiny
                    pieces = ((0, 512), (512, 1024), (1024, 1536), (1536, 1792), (1792, 2048)) \
                        if last_grp else ((0, 1024), (1024, 2048))
                    for c0, c1 in pieces:
                        nc.sync.dma_start(w_t[:, c0:c1], d["W1S"][G][:, c0:c1])
                elif G > 1:
                    nc.sync.dma_start(w_t[:], d["W1S"][G])
                if not bias_done[0]:
                    # rank-1 bias opens each f-region accumulation group:
                    # b1row-slice.T @ ones broadcasts b1 over the batch cols
                    for f in range(4):
                        nc.tensor.matmul(
                            ph1[f // 2][:, (f % 2) * 128:(f % 2) * 128 + 128],
                            b1r[:, f * 128:(f + 1) * 128], ones[:],
                            start=True, stop=False,
                        )
                    bias_done[0] = True
                for c in range(4):
                    t_idx = (G % 8) * 4 + c
                    last = last_grp and c == 3
                    for f in range(4):
                        nc.tensor.matmul(
                            ph1[f // 2][:, (f % 2) * 128:(f % 2) * 128 + 128],
                            w_t[:, c * 512 + f * 128: c * 512 + (f + 1) * 128],
                            hv[:, t_idx, :],
                            start=False, stop=last,
                        )
                if G == 8 * j and j + 2 < 8:
                    hs[j + 2] = a_phase(j + 2)
        # tail-only weights ship after the W1 stream so the last W1 byte
        # (the critical one) arrives earlier; W2 first (layer matmuls need
        # it ~1us after the last W1 byte), Wo last (head needs it ~5us later)
        for c0, c1 in ((0, 512), (512, 1024), (1024, 2048), (2048, 2048 + 4 * MOD)):
            nc.sync.dma_start(w2o[:, c0:c1], d["W2O"][:, c0:c1])

        # ---- tail: h1 drain + 4 layers + head, feature-major, 2 batch waves
        # of 64 samples. Per-wave activation tiles [128, 4 f-blocks x 64b]
        # keep the wave chains byte-disjoint (no false deps); every PSUM
        # drain is split across two banks so ScalarE (f0-f1) and VectorE
        # (f2-f3) drain in parallel while PE runs the other wave's matmuls.
        W0, W1W = WAVE0, BL - WAVE0
        wof, wsz = (0, WAVE0), (W0, W1W)
        cur = [None, None]
        for w in range(2):
            o, n = wof[w], wsz[w]
            cw0 = hp_pool.tile([128, 4 * n], bf16, tag=f"cw{w}")
            cur[w] = cw0
            for bk in range(2):
                src = ph1[bk][:].rearrange("p (f b) -> p f b", f=2)[:, :, o:o + n]
                dst = cw0[:].rearrange("p (f b) -> p f b", f=4)[:, 2 * bk:2 * bk + 2, :]
                if bk == 0:
                    nc.scalar.activation(dst, src, AF.Relu)
                else:
                    nc.vector.tensor_scalar_max(dst, src, 0.0)

        # 4 x (h = relu(W2 @ h + b2)): rank-1 bias opens each f accumulation
        osb = const.tile([MOD, BL], fp32)
        cw = cur
        for L in range(4):
            for w in range(2):
                n = wsz[w]
                pla = ps_l.tile([128, 2 * n], fp32, tag=f"pla{w}")
                plb = ps_l.tile([128, 2 * n], fp32, tag=f"plb{w}")
                for f in range(4):
                    pf = (pla if f < 2 else plb)[:, (f % 2) * n:(f % 2) * n + n]
                    nc.tensor.matmul(
                        pf, b2r[:, f * 128:(f + 1) * 128], ones[:, 0:n],
                        start=True, stop=False,
                    )
                    for k in range(4):
                        nc.tensor.matmul(
                            pf,
                            w2sb[:, k * 512 + f * 128: k * 512 + f * 128 + 128],
                            cw[w][:, k * n:(k + 1) * n],
                            start=False, stop=(k == 3),
                        )
                hq = hp_pool.tile([128, 4 * n], bf16, tag=f"hq{w}")
                nc.scalar.activation(hq[:, 0:2 * n], pla[:], AF.Relu)
                nc.vector.tensor_scalar_max(hq[:, 2 * n:4 * n], plb[:], 0.0)
                cw[w] = hq
        # head: out' = Wo @ h' + bo -> [97, 64] per wave; each wave's output
        # DMA fires as soon as its drain lands (head PSUM reuses the long-
        # drained h1 banks)
        for w in range(2):
            o, n = wof[w], wsz[w]
            pw = ps_h1.tile([128, 256], fp32, tag=("h1a" if w == 0 else "h1b"))
            po = pw[0:MOD, 0:n]
            nc.tensor.matmul(po, bor, ones[:, 0:n], start=True, stop=False)
            for k in range(4):
                nc.tensor.matmul(
                    po, wosb[:, k * MOD:(k + 1) * MOD],
                    cw[w][:, k * n:(k + 1) * n],
                    start=False, stop=(k == 3),
                )
            ow = osb[:, o:o + n]
            if w == 0:
                nc.scalar.copy(ow, po)
            else:
                nc.vector.tensor_copy(ow, po)
        nc.sync.dma_start(d["OUT"], osb[:])


def _host_prep(inputs):
    x = np.asarray(inputs["x"]).astype(np.int64)          # [B, T]
    emb = np.asarray(inputs["emb"], np.float32)           # [97, 512]
    Wf = np.asarray(inputs["Wf"], np.float32)
    bf = np.asarray(inputs["bf"], np.float32)
    Wb = np.asarray(inputs["Wb"], np.float32)
    bb = np.asarray(inputs["bb"], np.float32)
    W1 = np.asarray(inputs["W1"], np.float32)             # [512, 32768]
    b1 = np.asarray(inputs["b1"], np.float32)
    W2 = np.asarray(inputs["W2"], np.float32)
    b2 = np.asarray(inputs["b2"], np.float32)
    Wo = np.asarray(inputs["Wo"], np.float32)             # [97, 512]
    bo = np.asarray(inputs["bo"], np.float32)

    # fold embedding gather + input projection + bias:
    # a_d[:, b, s] = (Wd @ emb.T + bd)[:, idx] since onehot has exactly one 1
    WFE = np.ascontiguousarray(np.stack([
        (Wf @ emb.T + bf[:, None]).T,                     # [97, 512]
        (Wb @ emb.T + bb[:, None]).T,
    ]).transpose(1, 0, 2).reshape(MOD, 2 * HID)).astype(BF16)

    # per-core x rows, col = b*32 + s; fwd s = t, bwd s = reversed t; the
    # device replicates these over 97 partitions and compares with arange
    # to build the one-hot (values 0..96 are exact in bf16)
    xc = x.reshape(NCORES, BL, T)
    XR = np.concatenate([
        xc.reshape(NCORES, BL * T), xc[:, :, ::-1].reshape(NCORES, BL * T)
    ], axis=1).astype(BF16)                               # [NC, 8192]

    # per-partition columns: arange (one-hot compare), b1/b2 f-tiles, bo
    COL = np.zeros((128, 16), np.float32)
    COL[:, 0] = np.arange(128)
    COL[:, 1:5] = b1.reshape(4, 128).T
    COL[:, 5:9] = b2.reshape(4, 128).T
    COL[:MOD, 9] = bo
    BIAH = np.concatenate([b1, b2, bo]).astype(BF16)      # [1121]

    # W1 -> [64, 128, 2048]: group G = (d, m, tg) holds k-chunks for
    # t = 4*tg .. 4*tg+3 of direction d, hid-tile m, side by side
    # W1.T row layout is [t, d, m, p]-major (xcat col = t*1024 + d*512 + m*128)
    W1S = np.ascontiguousarray(
        W1.T.reshape(8, 4, 2, 4, 128, 512)       # [tg, tc, d, m, p, col]
        .transpose(2, 3, 0, 4, 1, 5)             # [d, m, tg, p, tc, col]
        .reshape(W1_GRP, 128, 2048)
    ).astype(BF16)
    W2S = np.ascontiguousarray(W2.T.reshape(4, 128, 512).transpose(1, 0, 2).reshape(128, 2048)).astype(BF16)
    WOS = np.ascontiguousarray(Wo.T.reshape(4, 128, MOD).transpose(1, 0, 2).reshape(128, 4 * MOD)).astype(BF16)
    W2O = np.concatenate([W2S, WOS], axis=1)

    shared = {"WFE": WFE, "W1S": W1S, "W2O": W2O, "COL": COL}
    in_maps = [
        dict(shared, BIA=np.concatenate([XR[c], BIAH]).reshape(1, -1))
        for c in range(NCORES)
    ]
    return in_maps


def _get_nc():
    if "nc" not in _CACHE:
        _CACHE["nc"] = _build()
    return _CACHE["nc"]


def kernel(**inputs):
    from concourse.bass_utils import run_bass_kernel_spmd

    nc = _get_nc()
    in_maps = _host_prep(inputs)
    res = run_bass_kernel_spmd(nc, in_maps, list(range(NCORES)))
    outs = [np.asarray(res.results[c]["OUT"], np.float32) for c in range(NCORES)]
    return np.ascontiguousarray(np.concatenate([o.T for o in outs], axis=0))  # [1024, 97]


# revision 55
# speedup vs baseline: 1.0768x; 1.0768x over previous
"""Trainium2 Bass kernel for the BiDirectionalRNN problem.

Math (matches the fp32 jax reference):
    e = emb[x]                                   # [B, T, 512]
    fwd:  h_t = relu(e_t @ Wf.T + bf + h_{t-1})  # fs[t]
    bwd over reversed e: bs[s]                   # generation order
    xcat = concat_t [fs[t], bs[t]]  -> [B, T*1024]
    h1 = relu(xcat @ W1.T + b1); 4x h = relu(h @ W2.T + b2); out = h @ Wo.T + bo

Strategy:
  * Data-parallel over batch: 1024/8 = 128 samples per NeuronCore.
  * Host folds embedding + input projection weights into per-direction
    tables WfeB = Wf @ emb.T + bf ([512, 97]). The device builds the
    one-hot of x on the fly (a rank-1 matmul replicates the x row over 97
    partitions, DVE is_equal against an arange column), then computes the
    per-step drive terms a = WfeB @ onehot with K=97 matmuls.
  * ScalarE copies each a-GEMM PSUM block into the scan layout
    [p, b*33 + s] (strided 3D AP), separator column = -1e30.
  * The whole 32-step recurrence h = relu(a + h_prev) runs as ONE DVE
    tensor_tensor_scan per (dir, hid-tile): state=(a add state) max 0,
    fp32 internal state; the separator resets state to 0 between chains.
  * W1 (64MB fp32) ships as bf16 (32MB/core) in 64 [128,2048] tiles,
    ordered (dir, m)-major so the GEMM can start right after the first
    scan; a-phases are software-pipelined two steps ahead of the groups
    that consume them. The [B,32768]@[32768,512] GEMM runs FEATURE-MAJOR:
    out[f, b] accumulates in one PSUM bank laid out as 4 f-tile column
    regions of 128 batch; lhsT = W1 k-chunk tiles [128k, 128f], rhs =
    strided scan-output views [128k, 128b]. This orientation needs no
    transposes at the tail and allows per-partition biases.
  * Tail: 4 x [512,512] layers + [97,512] head, all feature-major,
    pipelined in two 64-sample batch waves with per-wave activation tiles
    (byte-disjoint, no false deps): wave 1's matmuls run on PE while wave
    0's PSUM banks drain, so PE stays saturated through the tail. Every
    PSUM stage uses twin banks so ScalarE and VectorE drain in parallel
    (Tile serializes same-bank readers); biases enter PSUM via rank-1
    matmuls that open each accumulation group.
  * All const/small inputs ride in merged DMAs; the first two W1 groups
    are issued before them and the tail-only W2/Wo weights ship after the
    W1 stream (split so each piece's semaphore lands just before its
    layer matmuls), keeping the DMA engines gapless from first byte to
    the critical last W1 byte; the final four W1 groups are fetched in
    tapering chunk-aligned pieces to minimize the end latency.
"""

import numpy as np
import ml_dtypes

BF16 = ml_dtypes.bfloat16
FP8E3 = ml_dtypes.float8_e3m4   # TRN float8e3: 4 mantissa bits
W_SCALE = 256.0                 # W1 quant scale (folded out at the h1 drain)
DEQ = 1.0 / W_SCALE

MOD = 97
HID = 512
T = 32
B = 1024
NCORES = 8
BL = B // NCORES          # 128 batch per core
CL = T + 1                # chain length incl. separator column
FREE = BL * CL            # 4224 scan columns per tile
NEG = -1e30
W1_GRP = 64               # W1 DMA groups of 4 k-chunks (512KB each)
WAVE = 64                 # tail batch-wave size
WAVE0 = 64                # first-wave width (second wave = 128 - WAVE0)

_CACHE: dict = {}


def _build(reps=1):
    import concourse.tile as tile
    from concourse import bacc, mybir

    fp32 = mybir.dt.float32
    bf16 = mybir.dt.bfloat16

    nc = bacc.Bacc(
        "TRN2", target_bir_lowering=False, debug=False, num_devices=NCORES
    )

    d = {
        "WFE": nc.dram_tensor("WFE", [MOD, 2 * HID], bf16, kind="ExternalInput").ap(),
        "W1S": nc.dram_tensor("W1S", [W1_GRP, 128, 2048], mybir.dt.float8e3, kind="ExternalInput").ap(),
        "W2O": nc.dram_tensor("W2O", [128, 4 * 512 + 4 * MOD], bf16, kind="ExternalInput").ap(),
        "BIA": nc.dram_tensor("BIA", [1, 2 * BL * T + 1121], bf16, kind="ExternalInput").ap(),
        "COL": nc.dram_tensor("COL", [128, 16], fp32, kind="ExternalInput").ap(),
        "OUT": nc.dram_tensor("OUT", [MOD, BL], fp32, kind="ExternalOutput").ap(),
    }

    with tile.TileContext(nc) as tc:
        for _ in range(reps):
            _emit(tc, d, mybir)

    nc.compile()
    return nc


def _emit(tc, d, mybir):
    nc = tc.nc
    fp32 = mybir.dt.float32
    bf16 = mybir.dt.bfloat16
    AF = mybir.ActivationFunctionType
    ALU = mybir.AluOpType

    from contextlib import ExitStack

    with ExitStack() as ctx:
        const = ctx.enter_context(tc.tile_pool(name="const", bufs=1))
        a_pool = ctx.enter_context(tc.tile_pool(name="apool", bufs=2))
        h_pool = ctx.enter_context(tc.tile_pool(name="hpool", bufs=3))
        w1_pool = ctx.enter_context(tc.tile_pool(name="w1pool", bufs=24))
        hp_pool = ctx.enter_context(tc.tile_pool(name="hppool", bufs=5))
        ps_a = ctx.enter_context(tc.tile_pool(name="psa", bufs=2, space="PSUM"))
        ps_h1 = ctx.enter_context(tc.tile_pool(name="psh1", bufs=1, space="PSUM"))
        ps_l = ctx.enter_context(tc.tile_pool(name="psl", bufs=1, space="PSUM"))

        # ---- head: start the W1 stream before anything else ----
        # HWDGE descriptor generations serialize (~0.6us each); issuing the
        # first two W1 groups first keeps the DMA engines busy while the
        # const descriptors generate (W1 g0 isn't consumed until ~12us).
        fp8e3 = mybir.dt.float8e3
        w1_pre = {}
        for G in (0, 1):
            w_t = w1_pool.tile([128, 2048], fp8e3, tag="w_t")
            nc.sync.dma_start(w_t[:], d["W1S"][G])
            w1_pre[G] = w_t

        # ---- constants (merged DMAs to avoid early DMA-engine bubbles) ----
        wfe = const.tile([MOD, 2 * HID], bf16)
        nc.sync.dma_start(wfe[:], d["WFE"][:])
        bia = const.tile([1, 2 * BL * T + 1121], bf16)
        nc.sync.dma_start(bia[:], d["BIA"])
        xr = bia[:, 0:2 * BL * T]
        b1r = bia[:, 2 * BL * T:2 * BL * T + 512]
        b2r = bia[:, 2 * BL * T + 512:2 * BL * T + 1024]
        bor = bia[:, 2 * BL * T + 1024:2 * BL * T + 1121]
        col = const.tile([128, 16], fp32)
        nc.sync.dma_start(col[:], d["COL"])
        arn = col[:, 0:1]
        w2o = const.tile([128, 4 * 512 + 4 * MOD], bf16)
        w2sb = w2o[:, 0:2048]
        wosb = w2o[:, 2048:2048 + 4 * MOD]
        ones = const.tile([1, 128], bf16)
        nc.vector.memset(ones[:], 1.0)
        zero = const.tile([128, 1], bf16)
        nc.vector.memset(zero[:], 0.0)
        # one-hot of x, built on device: replicate the x row over 97
        # partitions with a rank-1 matmul, then compare against arange
        ohall = const.tile([MOD, 2 * BL * T], bf16)
        ohsb = [ohall[:, 0:BL * T], ohall[:, BL * T:2 * BL * T]]

        # ---- drive terms + scans + linear1, interleaved per j = dir*4 + m ----
        # a = WfeB @ onehot in 8 PSUM blocks of 16 chains; ScalarE lays each
        # block into the scan layout [p, b*33 + s] (strided 3D AP); the DVE
        # scan computes h = relu(a + h_prev) for all 128 chains in one
        # instruction; then the two W1 groups for this j stream in and
        # accumulate into psum_h1 (feature-major: 4 f-tile col regions).
        # W1 group order is (dir, m)-major so group G only needs scan j = G//8.
        # Two PSUM banks (f-tiles 0-1 / 2-3) so ScalarE and VectorE can drain
        # in parallel (Tile serializes same-bank readers).
        ph1a = ps_h1.tile([128, 256], fp32, tag="h1a")
        ph1b = ps_h1.tile([128, 256], fp32, tag="h1b")
        ph1 = [ph1a, ph1b]
        bias_done = [False]

        def a_phase(j):
            dd, m = j // 4, j % 4
            a_sb = a_pool.tile([128, FREE], bf16, tag="a")
            sep = a_sb[:].rearrange("p (b t) -> p b t", t=CL)[:, :, T]
            nc.vector.memset(sep, NEG)
            lhsT = wfe[:, dd * HID + m * 128: dd * HID + m * 128 + 128]
            for q in range(8):
                if m == 0:
                    px = ps_a.tile([128, 512], fp32, tag="pa")
                    nc.tensor.matmul(
                        px[:MOD, :], ones[:, 0:MOD],
                        xr[:, dd * BL * T + q * 512: dd * BL * T + (q + 1) * 512],
                        start=True, stop=True,
                    )
                    nc.vector.tensor_tensor(
                        ohsb[dd][:, q * 512:(q + 1) * 512], px[:MOD, :],
                        arn[:MOD, :].broadcast_to([MOD, 512]),
                        op=mybir.AluOpType.is_equal,
                    )
                pa = ps_a.tile([128, 512], fp32, tag="pa")
                nc.tensor.matmul(
                    pa[:], lhsT, ohsb[dd][:, q * 512:(q + 1) * 512],
                    start=True, stop=True,
                )
                av = a_sb[:].rearrange("p (b t) -> p b t", t=CL)[:, 16 * q:16 * q + 16, 0:T]
                nc.scalar.copy(av, pa[:].rearrange("p (b t) -> p b t", t=T))
            h_t = h_pool.tile([128, FREE], bf16, tag="h")
            nc.vector.tensor_tensor_scan(
                h_t[:], a_sb[:], zero[:].broadcast_to([128, FREE]),
                initial=0.0, op0=ALU.add, op1=ALU.max,
            )
            return h_t

        hs = {0: a_phase(0), 1: a_phase(1)}
        for j in range(8):
            hv = hs[j][:].rearrange("p (b t) -> p t b", t=CL)
            for G in range(8 * j, 8 * j + 8):
                w_t = w1_pre.pop(G, None)
                if w_t is None:
                    w_t = w1_pool.tile([128, 2048], fp8e3, tag="w_t")
                last_grp = G == W1_GRP - 1
                if G >= W1_GRP - 5:
                    # taper: fetch the final two groups in chunk-aligned
                    # pieces so each matmul only waits on its own slice and
                    # the post-stream PE backlog stays tiny
                    pieces = ((0, 512), (512, 1024), (1024, 1536), (1536, 1792), (1792, 2048)) \
                        if last_grp else ((0, 1024), (1024, 2048))
                    for c0, c1 in pieces:
                        nc.sync.dma_start(w_t[:, c0:c1], d["W1S"][G][:, c0:c1])
                elif G > 1:
                    nc.sync.dma_start(w_t[:], d["W1S"][G])
                if not bias_done[0]:
                    # rank-1 bias opens each f-region accumulation group:
                    # b1row-slice.T @ ones broadcasts b1 over the batch cols
                    for f in range(4):
                        nc.tensor.matmul(
                            ph1[f // 2][:, (f % 2) * 128:(f % 2) * 128 + 128],
                            b1r[:, f * 128:(f + 1) * 128], ones[:],
                            start=True, stop=False,
                        )
                    bias_done[0] = True
                for c in range(4):
                    t_idx = (G % 8) * 4 + c
                    last = last_grp and c == 3
                    for f in range(4):
                        nc.tensor.matmul(
                            ph1[f // 2][:, (f % 2) * 128:(f % 2) * 128 + 128],
                            w_t[:, c * 512 + f * 128: c * 512 + (f + 1) * 128],
                            hv[:, t_idx, :],
                            start=False, stop=last,
                        )
                if G == 8 * j and j + 2 < 8:
                    hs[j + 2] = a_phase(j + 2)
        # tail-only weights ship after the W1 stream so the last W1 byte
        # (the critical one) arrives earlier; W2 first (layer matmuls need
        # it ~1us after the last W1 byte), Wo last (head needs it ~5us later)
        for c0, c1 in ((0, 512), (512, 1024), (1024, 2048), (2048, 2048 + 4 * MOD)):
            nc.sync.dma_start(w2o[:, c0:c1], d["W2O"][:, c0:c1])

        # ---- tail: h1 drain + 4 layers + head, feature-major, 2 batch waves
        # of 64 samples. Per-wave activation tiles [128, 4 f-blocks x 64b]
        # keep the wave chains byte-disjoint (no false deps); every PSUM
        # drain is split across two banks so ScalarE (f0-f1) and VectorE
        # (f2-f3) drain in parallel while PE runs the other wave's matmuls.
        W0, W1W = WAVE0, BL - WAVE0
        wof, wsz = (0, WAVE0), (W0, W1W)
        cur = [None, None]
        for w in range(2):
            o, n = wof[w], wsz[w]
            cw0 = hp_pool.tile([128, 4 * n], bf16, tag=f"cw{w}")
            cur[w] = cw0
            for bk in range(2):
                src = ph1[bk][:].rearrange("p (f b) -> p f b", f=2)[:, :, o:o + n]
                dst = cw0[:].rearrange("p (f b) -> p f b", f=4)[:, 2 * bk:2 * bk + 2, :]
                if bk == 0:
                    nc.scalar.activation(dst, src, AF.Relu, scale=DEQ)
                else:
                    nc.vector.tensor_scalar(
                        dst, src, DEQ, 0.0, op0=ALU.mult, op1=ALU.max
                    )

        # 4 x (h = relu(W2 @ h + b2)): rank-1 bias opens each f accumulation
        osb = const.tile([MOD, BL], fp32)
        cw = cur
        for L in range(4):
            for w in range(2):
                n = wsz[w]
                pla = ps_l.tile([128, 2 * n], fp32, tag=f"pla{w}")
                plb = ps_l.tile([128, 2 * n], fp32, tag=f"plb{w}")
                for f in range(4):
                    pf = (pla if f < 2 else plb)[:, (f % 2) * n:(f % 2) * n + n]
                    nc.tensor.matmul(
                        pf, b2r[:, f * 128:(f + 1) * 128], ones[:, 0:n],
                        start=True, stop=False,
                    )
                    for k in range(4):
                        nc.tensor.matmul(
                            pf,
                            w2sb[:, k * 512 + f * 128: k * 512 + f * 128 + 128],
                            cw[w][:, k * n:(k + 1) * n],
                            start=False, stop=(k == 3),
                        )
                hq = hp_pool.tile([128, 4 * n], bf16, tag=f"hq{w}")
                nc.scalar.activation(hq[:, 0:2 * n], pla[:], AF.Relu)
                nc.vector.tensor_scalar_max(hq[:, 2 * n:4 * n], plb[:], 0.0)
                cw[w] = hq
        # head: out' = Wo @ h' + bo -> [97, 64] per wave; each wave's output
        # DMA fires as soon as its drain lands (head PSUM reuses the long-
        # drained h1 banks)
        for w in range(2):
            o, n = wof[w], wsz[w]
            pw = ps_h1.tile([128, 256], fp32, tag=("h1a" if w == 0 else "h1b"))
            po = pw[0:MOD, 0:n]
            nc.tensor.matmul(po, bor, ones[:, 0:n], start=True, stop=False)
            for k in range(4):
                nc.tensor.matmul(
                    po, wosb[:, k * MOD:(k + 1) * MOD],
                    cw[w][:, k * n:(k + 1) * n],
                    start=False, stop=(k == 3),
                )
            ow = osb[:, o:o + n]
            if w == 0:
                nc.scalar.copy(ow, po)
            else:
                nc.vector.tensor_copy(ow, po)
        nc.sync.dma_start(d["OUT"], osb[:])


def _host_prep(inputs):
    x = np.asarray(inputs["x"]).astype(np.int64)          # [B, T]
    emb = np.asarray(inputs["emb"], np.float32)           # [97, 512]
    Wf = np.asarray(inputs["Wf"], np.float32)
    bf = np.asarray(inputs["bf"], np.float32)
    Wb = np.asarray(inputs["Wb"], np.float32)
    bb = np.asarray(inputs["bb"], np.float32)
    W1 = np.asarray(inputs["W1"], np.float32)             # [512, 32768]
    b1 = np.asarray(inputs["b1"], np.float32)
    W2 = np.asarray(inputs["W2"], np.float32)
    b2 = np.asarray(inputs["b2"], np.float32)
    Wo = np.asarray(inputs["Wo"], np.float32)             # [97, 512]
    bo = np.asarray(inputs["bo"], np.float32)

    # fold embedding gather + input projection + bias:
    # a_d[:, b, s] = (Wd @ emb.T + bd)[:, idx] since onehot has exactly one 1
    WFE = np.ascontiguousarray(np.stack([
        (Wf @ emb.T + bf[:, None]).T,                     # [97, 512]
        (Wb @ emb.T + bb[:, None]).T,
    ]).transpose(1, 0, 2).reshape(MOD, 2 * HID)).astype(BF16)

    # per-core x rows, col = b*32 + s; fwd s = t, bwd s = reversed t; the
    # device replicates these over 97 partitions and compares with arange
    # to build the one-hot (values 0..96 are exact in bf16)
    xc = x.reshape(NCORES, BL, T)
    XR = np.concatenate([
        xc.reshape(NCORES, BL * T), xc[:, :, ::-1].reshape(NCORES, BL * T)
    ], axis=1).astype(BF16)                               # [NC, 8192]

    # per-partition columns: arange (one-hot compare), b1/b2 f-tiles, bo
    COL = np.zeros((128, 16), np.float32)
    COL[:, 0] = np.arange(128)
    COL[:, 1:5] = b1.reshape(4, 128).T
    COL[:, 5:9] = b2.reshape(4, 128).T
    COL[:MOD, 9] = bo
    BIAH = np.concatenate([b1 * W_SCALE, b2, bo]).astype(BF16)  # [1121]

    # e3m4 quantization of W1*2^8 with error feedback along t (adjacent-t
    # activations are highly correlated in the accumulating relu RNN, so
    # carrying the rounding error onto the next t's weight cancels most of
    # the weight-quant error); dequant 2^-8 is applied at the h1 drain
    Wr = (W1 * W_SCALE).reshape(512, T, 1024)
    carry = np.zeros((512, 1024), np.float32)
    Wq = np.empty((512, T, 1024), FP8E3)
    for t in range(T):
        v = Wr[:, t, :] + carry
        q = v.astype(FP8E3)
        carry = v - q.astype(np.float32)
        Wq[:, t, :] = q
    W1q = Wq.reshape(512, 32768)
    # W1 -> [64, 128, 2048]: group G = (d, m, tg) holds k-chunks for
    # t = 4*tg .. 4*tg+3 of direction d, hid-tile m, side by side
    # W1.T row layout is [t, d, m, p]-major (xcat col = t*1024 + d*512 + m*128)
    W1S = np.ascontiguousarray(
        W1q.T.reshape(8, 4, 2, 4, 128, 512)      # [tg, tc, d, m, p, col]
        .transpose(2, 3, 0, 4, 1, 5)             # [d, m, tg, p, tc, col]
        .reshape(W1_GRP, 128, 2048)
    )
    W2S = np.ascontiguousarray(W2.T.reshape(4, 128, 512).transpose(1, 0, 2).reshape(128, 2048)).astype(BF16)
    WOS = np.ascontiguousarray(Wo.T.reshape(4, 128, MOD).transpose(1, 0, 2).reshape(128, 4 * MOD)).astype(BF16)
    W2O = np.concatenate([W2S, WOS], axis=1)

    shared = {"WFE": WFE, "W1S": W1S, "W2O": W2O, "COL": COL}
    in_maps = [
        dict(shared, BIA=np.concatenate([XR[c], BIAH]).reshape(1, -1))
        for c in range(NCORES)
    ]
    return in_maps


def _get_nc():
    if "nc" not in _CACHE:
        _CACHE["nc"] = _build()
    return _CACHE["nc"]


def kernel(**inputs):
    from concourse.bass_utils import run_bass_kernel_spmd

    nc = _get_nc()
    in_maps = _host_prep(inputs)
    res = run_bass_kernel_spmd(nc, in_maps, list(range(NCORES)))
    outs = [np.asarray(res.results[c]["OUT"], np.float32) for c in range(NCORES)]
    return np.ascontiguousarray(np.concatenate([o.T for o in outs], axis=0))  # [1024, 97]
